# revision 11
# baseline (speedup 1.0000x reference)
"""Dual-head row-packed variant: head pairs share the 128-row PE array.

K^T and Q^T for heads (2hp, 2hp+1) are packed on partitions 0-63 / 64-127 of
the same SBUF tiles; the two 64-contraction score matmuls run concurrently in
different PE row-groups (tile_position auto-derived from base_partition) and
write different PSUM banks. Everything else matches kernel.py.
"""

import numpy as np
import ml_dtypes

B, H, S, DK = 4, 16, 2048, 64
NCORES = 8
HPC = H * B // NCORES
NPAIR = HPC // 2       # 4 head pairs
QT = 512
NQT = S // QT
KT = 128
NKT = S // KT
VE = DK + 1
SCALE = 1.0 / float(np.sqrt(DK))

_BF16 = ml_dtypes.bfloat16

_CACHE = {}


def _build_nc(reps=1):
    import concourse.mybir as mybir
    import concourse.tile as tile
    from concourse import bacc
    from concourse.masks import make_identity
    from contextlib import ExitStack

    dt = mybir.dt
    nc = bacc.Bacc()

    q2 = nc.declare_dram_parameter("q2", [NPAIR, 2 * DK, S], dt.bfloat16, isOutput=False)
    k2 = nc.declare_dram_parameter("k2", [NPAIR, 2 * DK, S], dt.bfloat16, isOutput=False)
    vex = nc.declare_dram_parameter("vex", [S, HPC, VE], dt.bfloat16, isOutput=False)
    maskT = nc.declare_dram_parameter("maskT", [S, S], dt.bfloat16, isOutput=False)
    out = nc.declare_dram_parameter("out", [HPC, S, DK], dt.float32, isOutput=True)

    with tile.TileContext(nc) as tc, ExitStack() as ctx:
        const = ctx.enter_context(tc.tile_pool(name="const", bufs=1))
        maskp = ctx.enter_context(tc.tile_pool(name="maskp", bufs=2))
        qp = ctx.enter_context(tc.tile_pool(name="qp", bufs=3))
        pp = ctx.enter_context(tc.tile_pool(name="pp", bufs=3))
        epi = ctx.enter_context(tc.tile_pool(name="epi", bufs=3))
        scps = ctx.enter_context(tc.tile_pool(name="scps", bufs=3, space="PSUM"))
        pvps = ctx.enter_context(tc.tile_pool(name="pvps", bufs=2, space="PSUM"))

        ident = const.tile([128, 128], dt.float32)
        make_identity(nc, ident)

        warm = const.tile([1, 2], dt.float32)
        nc.vector.memset(warm, 0.0)
        nc.scalar.activation(out=warm, in_=warm,
                             func=mybir.ActivationFunctionType.Exp)

        # K^T head pairs packed [128, pair, s]; pair 0 first for fast start
        k_sb = const.tile([2 * DK, NPAIR, S], dt.bfloat16)
        nc.sync.dma_start(out=k_sb[:, 0, :], in_=k2[0])
        nc.sync.dma_start(
            out=k_sb[:, 1:, :], in_=k2[1:].rearrange("h d s -> d h s")
        )

        v_sb = const.tile([KT, NKT, HPC, VE], dt.bfloat16)
        nc.sync.dma_start(out=v_sb, in_=vex.rearrange("(j p) h e -> p j h e", p=KT))

        for _rep in range(reps):
         for qt in range(NQT):
            m_sb = maskp.tile([KT, NKT, QT], dt.bfloat16)
            nc.sync.dma_start(
                out=m_sb,
                in_=maskT[:, qt * QT:(qt + 1) * QT].rearrange(
                    "(j p) q -> p j q", p=KT
                ),
            )
            for hp in range(NPAIR):
                q_sb = qp.tile([2 * DK, QT], dt.bfloat16)
                nc.sync.dma_start(out=q_sb, in_=q2[hp, :, qt * QT:(qt + 1) * QT])

                p_a = pp.tile([KT, NKT * QT], dt.bfloat16, tag="p_a")
                p_b = pp.tile([KT, NKT * QT], dt.bfloat16, tag="p_b")
                for i in range(NKT // 2):
                    j0 = 2 * i
                    for a, p_sb in ((0, p_a), (1, p_b)):
                        sc = scps.tile([KT, 2 * QT], dt.float32, tag="sc")
                        for u in range(2):
                            j = j0 + u
                            nc.tensor.matmul(
                                out=sc[:, u * QT:(u + 1) * QT],
                                lhsT=k_sb[64 * a:64 * a + 64, hp,
                                          j * KT:(j + 1) * KT],
                                rhs=q_sb[64 * a:64 * a + 64, :],
                                start=True,
                                stop=True,
                            )
                        nc.scalar.activation(
                            out=p_sb[:, j0 * QT:(j0 + 2) * QT],
                            in_=sc,
                            func=mybir.ActivationFunctionType.Exp,
                            scale=SCALE,
                        )
                        nc.vector.tensor_mul(
                            p_sb[:, j0 * QT:(j0 + 2) * QT],
                            p_sb[:, j0 * QT:(j0 + 2) * QT],
                            m_sb[:, j0:j0 + 2, :].rearrange("p a q -> p (a q)"),
                        )

                for a, p_sb in ((0, p_a), (1, p_b)):
                    h = 2 * hp + a
                    pv = pvps.tile([128, QT], dt.float32, tag="pv")
                    for j in range(NKT):
                        nc.tensor.matmul(
                            out=pv[0:VE, :],
                            lhsT=v_sb[:, j, h, :],
                            rhs=p_sb[:, j * QT:(j + 1) * QT],
                            start=(j == 0),
                            stop=(j == NKT - 1),
                        )

                    o_sb = epi.tile([VE, QT], dt.float32, tag="o_sb")
                    nc.vector.tensor_copy(o_sb, pv[0:VE, :])

                    tr = pv[:, 0:4 * VE]
                    for j in range(4):
                        nc.tensor.transpose(
                            out=tr[:, j * VE:(j + 1) * VE],
                            in_=o_sb[:, j * 128:(j + 1) * 128],
                            identity=ident[0:VE, 0:VE],
                        )
                    ot = epi.tile([128, 4, VE], dt.float32, tag="ot")
                    nc.vector.tensor_copy(ot, tr.rearrange("p (a e) -> p a e", e=VE))

                    rec = epi.tile([128, 4], dt.float32, tag="rec")
                    nc.vector.reciprocal(rec, ot[:, :, DK])

                    outf = epi.tile([128, 4, DK], dt.float32, tag="outf")
                    for j in range(4):
                        nc.vector.tensor_scalar_mul(
                            outf[:, j, :], ot[:, j, 0:DK], rec[:, j:j + 1]
                        )
                    nc.sync.dma_start(
                        out=out[h, qt * QT:(qt + 1) * QT, :].rearrange(
                            "(j p) d -> p j d", p=128
                        ),
                        in_=outf,
                    )
    nc.compile()
    return nc


def _get_nc(reps=1):
    key = ("nc", reps)
    if key not in _CACHE:
        _CACHE[key] = _build_nc(reps)
    return _CACHE[key]


def _prep_core_inputs(q, k, v, m, core):
    b = core // (H // HPC)
    h0 = (core % (H // HPC)) * HPC
    qs = q[b, h0:h0 + HPC].transpose(0, 2, 1).astype(_BF16)   # [8, DK, S]
    ks = k[b, h0:h0 + HPC].transpose(0, 2, 1).astype(_BF16)
    q2 = np.empty((NPAIR, 2 * DK, S), dtype=_BF16)
    k2 = np.empty((NPAIR, 2 * DK, S), dtype=_BF16)
    for hp in range(NPAIR):
        q2[hp, :DK] = qs[2 * hp]
        q2[hp, DK:] = qs[2 * hp + 1]
        k2[hp, :DK] = ks[2 * hp]
        k2[hp, DK:] = ks[2 * hp + 1]
    vex = np.ones((S, HPC, VE), dtype=_BF16)
    vex[:, :, :DK] = v[b, h0:h0 + HPC].transpose(1, 0, 2)
    mT = m[b, 0].T.astype(_BF16)
    return {"q2": q2, "k2": k2, "vex": vex, "maskT": np.ascontiguousarray(mT)}


def kernel(query, key, value, mask):
    from concourse.bass_utils import run_bass_kernel_spmd

    q = np.asarray(query, dtype=np.float32)
    k = np.asarray(key, dtype=np.float32)
    v = np.asarray(value, dtype=np.float32)
    m = np.asarray(mask)

    nc = _get_nc()
    in_maps = [_prep_core_inputs(q, k, v, m, c) for c in range(NCORES)]
    res = run_bass_kernel_spmd(nc, in_maps, list(range(NCORES))).results

    out = np.empty((B, H, S, DK), dtype=np.float32)
    for c in range(NCORES):
        b = c // (H // HPC)
        h0 = (c % (H // HPC)) * HPC
        out[b, h0:h0 + HPC] = res[c]["out"]
    return out


# revision 12
# speedup vs baseline: 2.1978x; 2.1978x over previous
"""Dual-head row-packed variant: head pairs share the 128-row PE array.

K^T and Q^T for heads (2hp, 2hp+1) are packed on partitions 0-63 / 64-127 of
the same SBUF tiles; the two 64-contraction score matmuls run concurrently in
different PE row-groups (tile_position auto-derived from base_partition) and
write different PSUM banks. Everything else matches kernel.py.
"""

import numpy as np
import ml_dtypes

B, H, S, DK = 4, 16, 2048, 64
NCORES = 8
HPC = H * B // NCORES
NPAIR = HPC // 2       # 4 head pairs
QT = 512
NQT = S // QT
KT = 128
NKT = S // KT
VE = DK + 1
SCALE = 1.0 / float(np.sqrt(DK))

_BF16 = ml_dtypes.bfloat16

_CACHE = {}


def _build_nc(reps=1):
    import concourse.mybir as mybir
    import concourse.tile as tile
    from concourse import bacc
    from concourse.masks import make_identity
    from contextlib import ExitStack

    dt = mybir.dt
    nc = bacc.Bacc()

    q2 = nc.declare_dram_parameter("q2", [NPAIR, 2 * DK, S], dt.bfloat16, isOutput=False)
    k2 = nc.declare_dram_parameter("k2", [NPAIR, 2 * DK, S], dt.bfloat16, isOutput=False)
    vex = nc.declare_dram_parameter("vex", [S, HPC, VE], dt.bfloat16, isOutput=False)
    maskT = nc.declare_dram_parameter("maskT", [S, S], dt.bfloat16, isOutput=False)
    out = nc.declare_dram_parameter("out", [HPC, S, DK], dt.float32, isOutput=True)

    with tile.TileContext(nc) as tc, ExitStack() as ctx:
        const = ctx.enter_context(tc.tile_pool(name="const", bufs=1))
        maskp = ctx.enter_context(tc.tile_pool(name="maskp", bufs=2))
        qp = ctx.enter_context(tc.tile_pool(name="qp", bufs=3))
        pp = ctx.enter_context(tc.tile_pool(name="pp", bufs=3))
        epi = ctx.enter_context(tc.tile_pool(name="epi", bufs=3))
        scps = ctx.enter_context(tc.tile_pool(name="scps", bufs=2, space="PSUM"))
        pvps = ctx.enter_context(tc.tile_pool(name="pvps", bufs=2, space="PSUM"))

        ident = const.tile([128, 128], dt.float32)
        make_identity(nc, ident)

        warm = const.tile([1, 2], dt.float32)
        nc.vector.memset(warm, 0.0)
        nc.scalar.activation(out=warm, in_=warm,
                             func=mybir.ActivationFunctionType.Exp)

        # K^T head pairs packed [128, pair, s]; pair 0 first for fast start
        k_sb = const.tile([2 * DK, NPAIR, S], dt.bfloat16)
        nc.sync.dma_start(out=k_sb[:, 0, :], in_=k2[0])
        nc.sync.dma_start(
            out=k_sb[:, 1:, :], in_=k2[1:].rearrange("h d s -> d h s")
        )

        v_sb = const.tile([KT, NKT, HPC, VE], dt.bfloat16)
        nc.sync.dma_start(out=v_sb, in_=vex.rearrange("(j p) h e -> p j h e", p=KT))

        for _rep in range(reps):
         for qt in range(NQT):
            m_sb = maskp.tile([KT, NKT, QT], dt.bfloat16)
            nc.sync.dma_start(
                out=m_sb,
                in_=maskT[:, qt * QT:(qt + 1) * QT].rearrange(
                    "(j p) q -> p j q", p=KT
                ),
            )
            for hp in range(NPAIR):
                q_sb = qp.tile([2 * DK, QT], dt.bfloat16)
                nc.sync.dma_start(out=q_sb, in_=q2[hp, :, qt * QT:(qt + 1) * QT])

                p_a = pp.tile([KT, NKT * QT], dt.bfloat16, tag="p_a")
                p_b = pp.tile([KT, NKT * QT], dt.bfloat16, tag="p_b")
                j0s = [0, 3, 6, 9, 12, 14]
                grps = [3, 3, 3, 3, 2, 2]
                for gi in range(6):
                    j0, grp = j0s[gi], grps[gi]
                    for a, p_sb in ((0, p_a), (1, p_b)):
                        sc = scps.tile([KT, 3 * QT], dt.float32, tag="sc")
                        for u in range(grp):
                            j = j0 + u
                            nc.tensor.matmul(
                                out=sc[:, u * QT:(u + 1) * QT],
                                lhsT=k_sb[64 * a:64 * a + 64, hp,
                                          j * KT:(j + 1) * KT],
                                rhs=q_sb[64 * a:64 * a + 64, :],
                                start=True,
                                stop=True,
                            )
                        nc.scalar.activation(
                            out=p_sb[:, j0 * QT:(j0 + grp) * QT],
                            in_=sc[:, 0:grp * QT],
                            func=mybir.ActivationFunctionType.Exp,
                            scale=SCALE,
                        )
                        nc.vector.tensor_mul(
                            p_sb[:, j0 * QT:(j0 + grp) * QT],
                            p_sb[:, j0 * QT:(j0 + grp) * QT],
                            m_sb[:, j0:j0 + grp, :].rearrange("p a q -> p (a q)"),
                        )

                for a, p_sb in ((0, p_a), (1, p_b)):
                    h = 2 * hp + a
                    pv = pvps.tile([128, QT], dt.float32, tag="pv")
                    for j in range(NKT):
                        nc.tensor.matmul(
                            out=pv[0:VE, :],
                            lhsT=v_sb[:, j, h, :],
                            rhs=p_sb[:, j * QT:(j + 1) * QT],
                            start=(j == 0),
                            stop=(j == NKT - 1),
                        )

                    o_sb = epi.tile([VE, QT], dt.float32, tag="o_sb")
                    nc.vector.tensor_copy(o_sb, pv[0:VE, :])

                    tr = pv[:, 0:4 * VE]
                    for j in range(4):
                        nc.tensor.transpose(
                            out=tr[:, j * VE:(j + 1) * VE],
                            in_=o_sb[:, j * 128:(j + 1) * 128],
                            identity=ident[0:VE, 0:VE],
                        )
                    ot = epi.tile([128, 4, VE], dt.float32, tag="ot")
                    nc.vector.tensor_copy(ot, tr.rearrange("p (a e) -> p a e", e=VE))

                    rec = epi.tile([128, 4], dt.float32, tag="rec")
                    nc.vector.reciprocal(rec, ot[:, :, DK])

                    outf = epi.tile([128, 4, DK], dt.float32, tag="outf")
                    for j in range(4):
                        nc.vector.tensor_scalar_mul(
                            outf[:, j, :], ot[:, j, 0:DK], rec[:, j:j + 1]
                        )
                    nc.sync.dma_start(
                        out=out[h, qt * QT:(qt + 1) * QT, :].rearrange(
                            "(j p) d -> p j d", p=128
                        ),
                        in_=outf,
                    )
    nc.compile()
    return nc


def _get_nc(reps=1):
    key = ("nc", reps)
    if key not in _CACHE:
        _CACHE[key] = _build_nc(reps)
    return _CACHE[key]


def _prep_core_inputs(q, k, v, m, core):
    b = core // (H // HPC)
    h0 = (core % (H // HPC)) * HPC
    qs = q[b, h0:h0 + HPC].transpose(0, 2, 1).astype(_BF16)   # [8, DK, S]
    ks = k[b, h0:h0 + HPC].transpose(0, 2, 1).astype(_BF16)
    q2 = np.empty((NPAIR, 2 * DK, S), dtype=_BF16)
    k2 = np.empty((NPAIR, 2 * DK, S), dtype=_BF16)
    for hp in range(NPAIR):
        q2[hp, :DK] = qs[2 * hp]
        q2[hp, DK:] = qs[2 * hp + 1]
        k2[hp, :DK] = ks[2 * hp]
        k2[hp, DK:] = ks[2 * hp + 1]
    vex = np.ones((S, HPC, VE), dtype=_BF16)
    vex[:, :, :DK] = v[b, h0:h0 + HPC].transpose(1, 0, 2)
    mT = m[b, 0].T.astype(_BF16)
    return {"q2": q2, "k2": k2, "vex": vex, "maskT": np.ascontiguousarray(mT)}


def kernel(query, key, value, mask):
    from concourse.bass_utils import run_bass_kernel_spmd

    q = np.asarray(query, dtype=np.float32)
    k = np.asarray(key, dtype=np.float32)
    v = np.asarray(value, dtype=np.float32)
    m = np.asarray(mask)

    nc = _get_nc()
    in_maps = [_prep_core_inputs(q, k, v, m, c) for c in range(NCORES)]
    res = run_bass_kernel_spmd(nc, in_maps, list(range(NCORES))).results

    out = np.empty((B, H, S, DK), dtype=np.float32)
    for c in range(NCORES):
        b = c // (H // HPC)
        h0 = (c % (H // HPC)) * HPC
        out[b, h0:h0 + HPC] = res[c]["out"]
    return out
